# revision 1
# baseline (speedup 1.0000x reference)
"""Trainium2 Bass kernel for nn_DCTHighPass.

Reference computation (per sample, 512x512 RGB image):
  gray = 0.299 R + 0.587 G + 0.114 B
  tiles = 8x8 blocks of gray (64x64 tiles, row-major (ti, tj))
  mag = |fft2(tile)|
  (buggy mask touches only batch 3:6 / fft rows 3:6 -> never sampled below)
  img = mag tiles stacked into [4096*8, 8]
  out = bilinear_resize(img, 512, 512)

Key reduction: the height downsample (32768 -> 512, factor 64) samples only
input rows 64i+31 and 64i+32 with weight 0.5 each, i.e. fft-row 7 of tile
(ti=i//8, tj=8*(i%8)+3) and fft-row 0 of tile (ti, tj=8*(i%8)+4).  fft rows
0/7 of an 8x8 real tile need only three 8-weight row reductions of the tile
(plain sum, cos, sin), followed by an 8-point DFT along columns.  The width
upsample (8 -> 512) is a fixed [8,512] matrix.  So per output row i:
  v = 0.5*(|DFT(cos/sin rowsums of tile tj3)| + |DFT(colsum of tile tj4)|)
  out[i, :] = v @ W8
Only image columns 64p+24 .. 64p+39 (p = i%8) are ever used.

Pipeline per sample (per core, batch of 8 samples):
  stage1 (PE):  x[128-row chunk] as stationary operand (128 needed cols),
                weights = per-channel (gray-coef x {1,cos,sin} x row-group)
                -> PSUM [128=(p,cc), 192=(q,type,tI_l)]
  stage2 (PE):  4 block-diag DFT matmuls -> [64=(p,k), 192] cos/sin x g0/g1
  magnitude (ACT/DVE): sqrt of sum of squares, x0.5 -> V [64=(p,k), 64=tI]
  stage3 (PE):  V (stationary) @ p-masked replicated W8 -> [64=tI, 512]
  out rows i = 8*tI + p, interleaved stores back to HBM.
"""

import sys

sys.path.insert(0, "/opt/trn_rl_repo")

import math
import numpy as np

from concourse import bacc
import concourse.mybir as mybir
from concourse.tile import TileContext
from concourse.bass_utils import run_bass_kernel_spmd

N_CORES = 8
B_FULL = 64
B_CORE = B_FULL // N_CORES  # 8 samples per core
H = W = 512
K = 8  # fft tile size
NQ = 4  # 128-row chunks per image
DT = mybir.dt.float32


# ----------------------------------------------------------------------------
# host-side constants
# ----------------------------------------------------------------------------
def _make_constants():
    j = np.arange(K)
    cosr = np.cos(2 * np.pi * j / K)
    sinr = np.sin(2 * np.pi * j / K)

    # wred [128, 144]: free = 48*ch + 16*type + tI_l
    # type 0: plain sum (A), 1: cos rowsum (Cr), 2: sin rowsum (Ci)
    coef = [0.299, 0.587, 0.114]
    wtypes = [np.ones(K), cosr, sinr]
    wred = np.zeros((128, 4 * 48), dtype=np.float32)
    r = np.arange(128)
    for ch in range(3):
        for ty in range(3):
            for t in range(16):
                rows = slice(8 * t, 8 * t + 8)
                wred[rows, 48 * ch + 16 * ty + t] = coef[ch] * wtypes[ty]
    for ty in range(3):
        for t in range(16):
            rows = slice(8 * t, 8 * t + 8)
            wred[rows, 144 + 16 * ty + t] = wtypes[ty]

    # dft matrices C[v,c] = cos(2pi v c/8), S[v,c] = sin(2pi v c/8)
    v = np.arange(K)
    C8 = np.cos(2 * np.pi * np.outer(v, j) / K).astype(np.float32)
    S8 = np.sin(2 * np.pi * np.outer(v, j) / K).astype(np.float32)

    # dftc [128, 320]: 5 blocks of 64 cols: [C*g0 | S*g0 | -S*g0 | C*g1 | S*g1]
    # partition = 16p + cc (cc in 0..15, g = cc//8); out col = 64*s + 8p + k
    dftc = np.zeros((128, 320), dtype=np.float32)
    for p in range(8):
        for cc in range(16):
            g, c = divmod(cc, 8)
            for k in range(8):
                cv, sv = C8[k, c], S8[k, c]
                if g == 0:
                    dftc[16 * p + cc, 0 + 8 * p + k] = cv
                    dftc[16 * p + cc, 64 + 8 * p + k] = sv
                    dftc[16 * p + cc, 128 + 8 * p + k] = -sv
                else:
                    dftc[16 * p + cc, 192 + 8 * p + k] = cv
                    dftc[16 * p + cc, 256 + 8 * p + k] = sv

    # W8 [8, 512]: bilinear width resize 8 -> 512 (align_corners=False)
    src = (np.arange(W) + 0.5) * (K / W) - 0.5
    src = np.clip(src, 0.0, K - 1.0)
    i0 = np.floor(src).astype(np.int64)
    i1 = np.minimum(i0 + 1, K - 1)
    fr = (src - i0).astype(np.float32)
    W8 = np.zeros((K, W), dtype=np.float32)
    for jj in range(W):
        W8[i0[jj], jj] += 1.0 - fr[jj]
        W8[i1[jj], jj] += fr[jj]

    # wrep [64, 8*512]: block p holds W8 on partitions 8p..8p+7, zero elsewhere
    wrep = np.zeros((64, 8 * W), dtype=np.float32)
    for p in range(8):
        wrep[8 * p : 8 * p + 8, W * p : W * p + W] = W8

    return wred, dftc, wrep


_WRED, _DFTC, _WREP = _make_constants()


# ----------------------------------------------------------------------------
# bass program (identical on all cores; per-core inputs differ)
# ----------------------------------------------------------------------------
CFG = dict(ps1_bufs=3, psab_bufs=2, ps3_bufs=3, load_split=2, store_split=1,
           copy_pat="avva", xin_bufs=3, mid_bufs=2, wrep_pool=True,
           merge_stores=True, load_mode="sparse", loads_only=False, skip_stores=False,
           gray_pre=True, wide_ps3=False, split_ch=False)


def _build_program(repeat=1):
    nc = bacc.Bacc()

    xs = nc.declare_dram_parameter("xs", [B_CORE, 3, H, W], DT, isOutput=False)
    wred_d = nc.declare_dram_parameter("wred", [128, 192], DT, isOutput=False)
    dftc_d = nc.declare_dram_parameter("dftc", [128, 320], DT, isOutput=False)
    wrep_d = nc.declare_dram_parameter("wrep", [64, 8 * W], mybir.dt.float32r, isOutput=False)
    ys = nc.declare_dram_parameter("ys", [B_CORE, 1, H, W], DT, isOutput=True)

    with TileContext(nc) as tc:
        with (
            tc.tile_pool(name="consts", bufs=1) as cpool,
            tc.tile_pool(name="xin", bufs=CFG["xin_bufs"]) as xpool,
            tc.tile_pool(name="xa", bufs=2) as xapool,
            tc.tile_pool(name="mid", bufs=CFG["mid_bufs"]) as mpool,
            tc.tile_pool(name="outp", bufs=CFG.get("outp_bufs", 2)) as opool,
            tc.tile_pool(name="ps1", bufs=CFG["ps1_bufs"], space="PSUM") as ps1pool,
            tc.tile_pool(name="ps2", bufs=CFG["psab_bufs"], space="PSUM") as ps2pool,
            tc.tile_pool(name="ps3", bufs=CFG["ps3_bufs"], space="PSUM") as ps3pool,
        ):
            wred_sb = cpool.tile([128, 192], DT, tag="wred")
            nc.sync.dma_start(wred_sb[:], wred_d[:])
            dftc_sb = cpool.tile([128, 320], DT, tag="dftc")
            nc.sync.dma_start(dftc_sb[:], dftc_d[:])
            wrep_sb = cpool.tile([64, 8 * W], mybir.dt.float32r, tag="wrep")
            (nc.gpsimd if CFG["wrep_pool"] else nc.sync).dma_start(
                wrep_sb[:], wrep_d[:]
            )

            rep_ctx = tc.For_i(0, repeat, 1) if repeat > 1 else None
            if rep_ctx is not None:
                rep_ctx.__enter__()
            for bg2 in range(B_CORE // 2):
                # ---- two samples per iteration: stage2/3 run at 128-wide ----
                xn = []
                li = 0
                for smp in range(2):
                    bg = 2 * bg2 + smp
                    if CFG["split_ch"]:
                        # one tile per channel; G first so the gray chain can
                        # start after 4 loads instead of 12
                        chtiles = {}
                        xsrc = xs[bg].rearrange(
                            "ch (q p) (g c) -> p ch q g c", p=128, g=8
                        )
                        for ch in (1, 0, 2):
                            t = xpool.tile([128, NQ * 128], DT, tag=f"xc{smp}{ch}")
                            tv = t.rearrange("p (q g c) -> p q g c", q=NQ, g=8)
                            for q in range(NQ):
                                eng = (
                                    nc.scalar
                                    if (li % 6) < CFG["load_split"]
                                    else nc.sync
                                )
                                eng.dma_start(tv[:, q], xsrc[:, ch, q, :, 24:40])
                                li += 1
                            chtiles[ch] = t
                        xn.append(chtiles)
                    else:
                        xneed = xpool.tile([128, 3 * NQ * 128], DT, tag=f"xn{smp}")
                        xneedv = xneed.rearrange(
                            "p (ch q g c) -> p ch q g c", ch=3, q=NQ, g=8
                        )
                        xsrc = xs[bg].rearrange(
                            "ch (q p) (g c) -> p ch q g c", p=128, g=8
                        )
                        for ch in range(3):
                            for q in range(NQ):
                                eng = (
                                    nc.scalar
                                    if (li % 6) < CFG["load_split"]
                                    else nc.sync
                                )
                                eng.dma_start(
                                    xneedv[:, ch, q], xsrc[:, ch, q, :, 24:40]
                                )
                                li += 1
                        xn.append(xneed.rearrange("p (blk c) -> p blk c", c=128))

                # ---- stage 1: row reductions per sample ----
                rhs2 = mpool.tile([128, 2 * 192], DT, tag="rhs2")
                if CFG["gray_pre"]:
                    # gray = 0.299 R + 0.587 G + 0.114 B on ACT/DVE, then a
                    # single unscaled reduction matmul per (smp, q)
                    xgray = mpool.tile([128, 2 * 512], DT, tag="xgray")
                    for smp in range(2):
                        xg = xgray[:, 512 * smp : 512 * smp + 512]
                        if CFG["split_ch"]:
                            rch = xn[smp][0][:]
                            gch = xn[smp][1][:]
                            bch = xn[smp][2][:]
                        else:
                            xr = xn[smp].rearrange("p blk c -> p (blk c)")
                            rch, gch, bch = (
                                xr[:, 0:512], xr[:, 512:1024], xr[:, 1024:1536]
                            )
                        t1 = mpool.tile([128, 512], DT, tag=f"t1{smp}")
                        nc.scalar.activation(
                            t1[:], gch,
                            mybir.ActivationFunctionType.Copy, scale=0.587,
                        )
                        t2 = mpool.tile([128, 512], DT, tag=f"t2{smp}")
                        nc.vector.scalar_tensor_tensor(
                            t2[:], rch, 0.299, t1[:],
                            mybir.AluOpType.mult, mybir.AluOpType.add,
                        )
                        nc.vector.scalar_tensor_tensor(
                            xg, bch, 0.114, t2[:],
                            mybir.AluOpType.mult, mybir.AluOpType.add,
                        )
                    xgv = xgray.rearrange("p (sq c) -> p sq c", c=128)
                    for smp in range(2):
                        ps1 = ps1pool.tile([128, 192], DT, tag="ps1")
                        for q in range(NQ):
                            nc.tensor.matmul(
                                ps1[:, 48 * q : 48 * q + 48],
                                xgv[:, 4 * smp + q],
                                wred_sb[:, 144:192],
                                start=True, stop=True,
                            )
                        nc.vector.tensor_copy(
                            rhs2[:, 192 * smp : 192 * smp + 192], ps1[:]
                        )
                else:
                    for smp in range(2):
                        ps1 = ps1pool.tile([128, 192], DT, tag="ps1")
                        for q in range(NQ):
                            for ch in range(3):
                                nc.tensor.matmul(
                                    ps1[:, 48 * q : 48 * q + 48],
                                    xn[smp][:, NQ * ch + q],
                                    wred_sb[:, 48 * ch : 48 * ch + 48],
                                    start=(ch == 0),
                                    stop=(ch == 2),
                                )
                        nc.vector.tensor_copy(
                            rhs2[:, 192 * smp : 192 * smp + 192], ps1[:]
                        )

                # ---- stage 2: DFT + height-blend fused via PSUM accumulation
                # psQ [64=(p,k), 512] = [R3 | I3 | R4 | I4] blocks of (smp, q, tI_l)
                rhs2v = rhs2.rearrange("p (s q blk) -> p s q blk", s=2, q=NQ)
                selA = rhs2v[:, :, :, 0:16]
                selCr = rhs2v[:, :, :, 16:32]
                selCi = rhs2v[:, :, :, 32:48]
                psQ = ps2pool.tile([64, 512], DT, tag="psQ")
                C0 = dftc_sb[:, 0:64]
                S0 = dftc_sb[:, 64:128]
                S0n = dftc_sb[:, 128:192]
                C1 = dftc_sb[:, 192:256]
                S1 = dftc_sb[:, 256:320]
                nc.tensor.matmul(psQ[:, 0:128], C0, selCr, start=True, stop=False)
                nc.tensor.matmul(psQ[:, 0:128], S0, selCi, start=False, stop=True)
                nc.tensor.matmul(psQ[:, 128:256], C0, selCi, start=True, stop=False)
                nc.tensor.matmul(psQ[:, 128:256], S0n, selCr, start=False, stop=True)
                nc.tensor.matmul(psQ[:, 256:384], C1, selA, start=True, stop=True)
                nc.tensor.matmul(psQ[:, 384:512], S1, selA, start=True, stop=True)

                # magnitudes: m = 0.5*sqrt(re^2 + im^2), [64, 128] each
                Sq = mybir.ActivationFunctionType.Square
                p3 = mpool.tile([64, 128], DT, tag="p3")
                nc.scalar.activation(p3[:], psQ[:, 0:128], Sq)
                q3 = mpool.tile([64, 128], DT, tag="q3")
                nc.scalar.activation(q3[:], psQ[:, 128:256], Sq)
                s3 = mpool.tile([64, 128], DT, tag="s3")
                nc.vector.tensor_add(s3[:], p3[:], q3[:])
                m3 = mpool.tile([64, 128], DT, tag="m3")
                nc.scalar.activation(
                    m3[:], s3[:], mybir.ActivationFunctionType.Sqrt, scale=0.25
                )
                p4 = mpool.tile([64, 128], DT, tag="p4")
                nc.scalar.activation(p4[:], psQ[:, 256:384], Sq)
                q4 = mpool.tile([64, 128], DT, tag="q4")
                nc.scalar.activation(q4[:], psQ[:, 384:512], Sq)
                s4 = mpool.tile([64, 128], DT, tag="s4")
                nc.vector.tensor_add(s4[:], p4[:], q4[:])
                m4 = mpool.tile([64, 128], DT, tag="m4")
                nc.scalar.activation(
                    m4[:], s4[:], mybir.ActivationFunctionType.Sqrt, scale=0.25
                )
                vt = mpool.tile([64, 128], mybir.dt.float32r, tag="vt")
                nc.vector.tensor_add(vt[:], m3[:], m4[:])

                # ---- stage 3: width resize; out partitions = (smp, tI) ----
                outse = opool.tile([128, NQ * W], DT, tag="outse")
                outso = opool.tile([128, NQ * W], DT, tag="outso")
                if CFG["wide_ps3"]:
                    # two stage-3 matmuls share one 2-bank PSUM tile; one
                    # [128,1024] copy drains both (half the copy instructions)
                    for pp in range(4):
                        ps3w = ps3pool.tile([128, 2 * W], DT, tag="ps3w")
                        for half in range(2):
                            p = 2 * pp + half
                            nc.tensor.matmul(
                                ps3w[:, W * half : W * half + W],
                                vt[:],
                                wrep_sb[:, W * p : W * p + W],
                                start=True, stop=True,
                            )
                        # p=2pp -> even (outse, v=pp), p=2pp+1 -> odd (outso, v=pp)
                        dste = outse[:, W * pp : W * pp + W]
                        dsto = outso[:, W * pp : W * pp + W]
                        if CFG["copy_pat"][pp % 4] == "v":
                            nc.vector.tensor_copy(dste, ps3w[:, 0:W])
                            nc.scalar.copy(dsto, ps3w[:, W : 2 * W])
                        else:
                            nc.scalar.copy(dste, ps3w[:, 0:W])
                            nc.vector.tensor_copy(dsto, ps3w[:, W : 2 * W])
                else:
                    for p in range(8):
                        v, e2 = divmod(p, 2)
                        ps3 = ps3pool.tile([128, W], DT, tag="ps3")
                        nc.tensor.matmul(
                            ps3[:],
                            vt[:],
                            wrep_sb[:, W * p : W * p + W],
                            start=True, stop=True,
                        )
                        dst = (outso if e2 else outse)[:, W * v : W * v + W]
                        if CFG["copy_pat"][p % 4] == "v":
                            nc.vector.tensor_copy(dst, ps3[:])
                        else:
                            nc.scalar.copy(dst, ps3[:])

                # merged pair stores per sample: rows 8t+e and 8t+e+4
                for smp in range(2) if not CFG["skip_stores"] else []:
                    bg = 2 * bg2 + smp
                    yr2 = ys[bg, 0].rearrange(
                        "(t h e) j -> e t h j", h=2, e=4
                    )  # i = 8t + 4h + e
                    for e in range(4):
                        v0, e2 = divmod(e, 2)
                        src = outso if e2 else outse
                        sap = src.rearrange("p (u v j) -> p u v j", u=2, v=2)[
                            64 * smp : 64 * smp + 64, :, v0
                        ]
                        eng = nc.sync if e < CFG["store_split"] else nc.gpsimd
                        eng.dma_start(yr2[e], sap)

            if rep_ctx is not None:
                rep_ctx.__exit__(None, None, None)

    nc.compile()
    return nc


_NC = None


def _get_program():
    global _NC
    if _NC is None:
        _NC = _build_program()
    return _NC


def kernel(x: np.ndarray) -> np.ndarray:
    assert x.shape == (B_FULL, 3, H, W), x.shape
    x = np.ascontiguousarray(x, dtype=np.float32)
    nc = _get_program()
    in_maps = []
    for c in range(N_CORES):
        in_maps.append(
            {
                "xs": x[c * B_CORE : (c + 1) * B_CORE],
                "wred": _WRED,
                "dftc": _DFTC,
                "wrep": _WREP,
            }
        )
    res = run_bass_kernel_spmd(nc, in_maps, core_ids=list(range(N_CORES)))
    out = np.concatenate([res.results[c]["ys"] for c in range(N_CORES)], axis=0)
    return out



# revision 8
# speedup vs baseline: 2.3910x; 2.3910x over previous
"""Trainium2 Bass kernel for nn_DCTHighPass.

Reference computation (per sample, 512x512 RGB image):
  gray = 0.299 R + 0.587 G + 0.114 B
  tiles = 8x8 blocks of gray (64x64 tiles, row-major (ti, tj))
  mag = |fft2(tile)|
  (buggy mask touches only batch 3:6 / fft rows 3:6 -> never sampled below)
  img = mag tiles stacked into [4096*8, 8]
  out = bilinear_resize(img, 512, 512)

Key reduction: the height downsample (32768 -> 512, factor 64) samples only
input rows 64i+31 and 64i+32 with weight 0.5 each, i.e. fft-row 7 of tile
(ti=i//8, tj=8*(i%8)+3) and fft-row 0 of tile (ti, tj=8*(i%8)+4).  fft rows
0/7 of an 8x8 real tile need only three 8-weight row reductions of the tile
(plain sum, cos, sin), followed by an 8-point DFT along columns.  The width
upsample (8 -> 512) is a fixed [8,512] matrix.  So per output row i:
  v = 0.5*(|DFT(cos/sin rowsums of tile tj3)| + |DFT(colsum of tile tj4)|)
  out[i, :] = v @ W8
Only image columns 64p+24 .. 64p+39 (p = i%8) are ever used.

v2 layout strategy (DMA was 75% busy in v1):
  - host gathers the 128 needed columns into a dense [B, 128p, 3ch*4q*128c]
    bf16 array, so device loads are contiguous 3KB-per-partition DMAs
    (v1's strided 64B descriptors ran at ~9B/ns/engine vs 22.5 peak);
  - stage-1/2 matmuls run in bf16 (fp32 pays 4 cycles/row on PE, bf16 1);
  - stage 3 stays fp32r (free dim 512 -> 1 cycle/row, full precision);
  - output is written bf16 (halves store traffic), one merged 8KB-per-
    partition store per sample; host upcasts to fp32.

Pipeline per 2 samples (per core, batch of 8 samples):
  load  xin [128, 1536] bf16 per sample
  gray (ACT/DVE, bf16): g = .299R + .587G + .114B -> xgray [128, 2*512]
  stage1 (PE, bf16): per (smp, q): xgray chunk stationary, wred [128,48]
         moving -> ps1 [128=(p,cc), 48=(type,tI_l)] -> rhs2 bf16
  stage2 (PE, bf16): 6 block-diag DFT matmuls -> psQ [64=(p,k), 512]
  magnitude (ACT/DVE): 0.5*sqrt(re^2+im^2) sums -> vt [64, 128] fp32r
  stage3 (PE, fp32r): vt stationary @ p-masked replicated W8 -> 8x [128, 512]
  drain PSUM -> out_all [128=(smp,tI), 8p*512j] bf16 (DVE/ACT/Pool copies)
  store ys rows 8*tI+p: one [64, 4096] DMA per sample (8KB contiguous rows)
"""

import sys

sys.path.insert(0, "/opt/trn_rl_repo")

import numpy as np
import ml_dtypes

from concourse import bacc
import concourse.mybir as mybir
from concourse.tile import TileContext
from concourse.bass_utils import run_bass_kernel_spmd

N_CORES = 8
B_FULL = 64
B_CORE = B_FULL // N_CORES  # 8 samples per core
H = W = 512
K = 8  # fft tile size
NQ = 4  # 128-row chunks per image
F32 = mybir.dt.float32
BF16 = mybir.dt.bfloat16
F32R = mybir.dt.float32r

# image columns ever sampled by the width resize: 64p+24 .. 64p+39
_COLS = np.concatenate([np.arange(64 * p + 24, 64 * p + 40) for p in range(K)])


# ----------------------------------------------------------------------------
# host-side constants
# ----------------------------------------------------------------------------
def _make_constants():
    j = np.arange(K)
    cosr = np.cos(2 * np.pi * j / K)
    sinr = np.sin(2 * np.pi * j / K)

    # wred [128, 144]: free = 48*ch + 16*type + tI_l, gray coef folded in;
    # type 0: plain sum (A), 1: cos rowsum (Cr), 2: sin rowsum (Ci);
    # partition = 8*tI_l + row
    coef = [0.299, 0.587, 0.114]
    wtypes = [np.ones(K), cosr, sinr]
    wred = np.zeros((128, 144), dtype=np.float32)
    for ch in range(3):
        for ty in range(3):
            for t in range(16):
                wred[8 * t : 8 * t + 8, 48 * ch + 16 * ty + t] = (
                    coef[ch] * wtypes[ty]
                )

    # dft matrices C[v,c] = cos(2pi v c/8), S[v,c] = sin(2pi v c/8)
    v = np.arange(K)
    C8 = np.cos(2 * np.pi * np.outer(v, j) / K).astype(np.float32)
    S8 = np.sin(2 * np.pi * np.outer(v, j) / K).astype(np.float32)

    # dftc [128, 320]: 5 blocks of 64 cols: [C*g0 | S*g0 | -S*g0 | C*g1 | S*g1]
    # partition = 16p + cc (cc in 0..15, g = cc//8); out col = 64*s + 8p + k
    dftc = np.zeros((128, 320), dtype=np.float32)
    for p in range(8):
        for cc in range(16):
            g, c = divmod(cc, 8)
            for k in range(8):
                cv, sv = C8[k, c], S8[k, c]
                if g == 0:
                    dftc[16 * p + cc, 0 + 8 * p + k] = cv
                    dftc[16 * p + cc, 64 + 8 * p + k] = sv
                    dftc[16 * p + cc, 128 + 8 * p + k] = -sv
                else:
                    dftc[16 * p + cc, 192 + 8 * p + k] = cv
                    dftc[16 * p + cc, 256 + 8 * p + k] = sv

    # W8 [8, 512]: bilinear width resize 8 -> 512 (align_corners=False)
    src = (np.arange(W) + 0.5) * (K / W) - 0.5
    src = np.clip(src, 0.0, K - 1.0)
    i0 = np.floor(src).astype(np.int64)
    i1 = np.minimum(i0 + 1, K - 1)
    fr = (src - i0).astype(np.float32)
    W8 = np.zeros((K, W), dtype=np.float32)
    for jj in range(W):
        W8[i0[jj], jj] += 1.0 - fr[jj]
        W8[i1[jj], jj] += fr[jj]

    # wrep [64, 8*512]: block p holds W8 on partitions 8p..8p+7, zero elsewhere
    wrep = np.zeros((64, 8 * W), dtype=np.float32)
    for p in range(8):
        wrep[8 * p : 8 * p + 8, W * p : W * p + W] = W8

    return (
        wred.astype(ml_dtypes.bfloat16),
        dftc.astype(ml_dtypes.bfloat16),
        wrep,
    )


_WRED, _DFTC, _WREP = _make_constants()


# ----------------------------------------------------------------------------
# bass program (identical on all cores; per-core inputs differ)
# ----------------------------------------------------------------------------
CFG = dict(xin_bufs=3, mid_bufs=3, out_bufs=3, ps1_bufs=2, ps2_bufs=2,
           ps3_bufs=3, copy_pat="avavavav")


def _build_program(repeat=1):
    nc = bacc.Bacc()

    xs = nc.declare_dram_parameter("xs", [B_CORE, 128, 3 * NQ * 128], BF16, isOutput=False)
    wred_d = nc.declare_dram_parameter("wred", [128, 144], BF16, isOutput=False)
    dftc_d = nc.declare_dram_parameter("dftc", [128, 320], BF16, isOutput=False)
    wrep_d = nc.declare_dram_parameter("wrep", [64, 8 * W], F32R, isOutput=False)
    ys = nc.declare_dram_parameter("ys", [B_CORE, 1, H, W], BF16, isOutput=True)

    with TileContext(nc) as tc:
        with (
            tc.tile_pool(name="consts", bufs=1) as cpool,
            tc.tile_pool(name="xin", bufs=CFG["xin_bufs"]) as xpool,
            tc.tile_pool(name="mid", bufs=CFG["mid_bufs"]) as mpool,
            tc.tile_pool(name="outp", bufs=CFG["out_bufs"]) as opool,
            tc.tile_pool(name="ps1", bufs=CFG["ps1_bufs"], space="PSUM") as ps1pool,
            tc.tile_pool(name="ps2", bufs=CFG["ps2_bufs"], space="PSUM") as ps2pool,
            tc.tile_pool(name="ps3", bufs=CFG["ps3_bufs"], space="PSUM") as ps3pool,
        ):
            wred_sb = cpool.tile([128, 144], BF16, tag="wred")
            nc.sync.dma_start(wred_sb[:], wred_d[:])
            dftc_sb = cpool.tile([128, 320], BF16, tag="dftc")
            nc.sync.dma_start(dftc_sb[:], dftc_d[:])
            wrep_sb = cpool.tile([64, 8 * W], F32R, tag="wrep")
            nc.gpsimd.dma_start(wrep_sb[:], wrep_d[:])

            rep_ctx = tc.For_i(0, repeat, 1) if repeat > 1 else None
            if rep_ctx is not None:
                rep_ctx.__enter__()
            for bg2 in range(B_CORE // 2):
                # ---- loads: one contiguous [128, 1536] bf16 DMA per sample
                xn = []
                for smp in range(2):
                    bg = 2 * bg2 + smp
                    xin = xpool.tile([128, 3 * NQ * 128], BF16, tag=f"xn{smp}")
                    nc.gpsimd.dma_start(xin[:], xs[bg])
                    xn.append(xin.rearrange("p (ch q c) -> p ch q c", ch=3, q=NQ))

                # ---- stage 1: gray folded into channel-accumulated row
                # reductions -> rhs2 [128, 2*192] bf16
                rhs2 = mpool.tile([128, 2 * 192], BF16, tag="rhs2")
                for smp in range(2):
                    ps1 = ps1pool.tile([128, 192], F32, tag="ps1")
                    for q in range(NQ):
                        for ch in range(3):
                            nc.tensor.matmul(
                                ps1[:, 48 * q : 48 * q + 48],
                                xn[smp][:, ch, q],
                                wred_sb[:, 48 * ch : 48 * ch + 48],
                                start=(ch == 0), stop=(ch == 2),
                            )
                    nc.vector.tensor_copy(
                        rhs2[:, 192 * smp : 192 * smp + 192], ps1[:]
                    )

                # ---- stage 2: DFT + height-blend fused via PSUM accumulation
                # psQ [64=(p,k), 512] = [R3 | I3 | R4 | I4] blocks of (smp,q,tI_l)
                rhs2v = rhs2.rearrange("p (s q blk) -> p s q blk", s=2, q=NQ)
                selA = rhs2v[:, :, :, 0:16]
                selCr = rhs2v[:, :, :, 16:32]
                selCi = rhs2v[:, :, :, 32:48]
                psQ = ps2pool.tile([64, 512], F32, tag="psQ")
                C0 = dftc_sb[:, 0:64]
                S0 = dftc_sb[:, 64:128]
                S0n = dftc_sb[:, 128:192]
                C1 = dftc_sb[:, 192:256]
                S1 = dftc_sb[:, 256:320]
                nc.tensor.matmul(psQ[:, 0:128], C0, selCr, start=True, stop=False)
                nc.tensor.matmul(psQ[:, 0:128], S0, selCi, start=False, stop=True)
                nc.tensor.matmul(psQ[:, 128:256], C0, selCi, start=True, stop=False)
                nc.tensor.matmul(psQ[:, 128:256], S0n, selCr, start=False, stop=True)
                nc.tensor.matmul(psQ[:, 256:384], C1, selA, start=True, stop=True)
                nc.tensor.matmul(psQ[:, 384:512], S1, selA, start=True, stop=True)

                # magnitudes: m = 0.5*sqrt(re^2 + im^2); psQ blocks are
                # [R3 | I3 | R4 | I4], pair-summed via strided APs
                sq = mpool.tile([64, 512], F32, tag="sq")
                nc.scalar.activation(
                    sq[:], psQ[:], mybir.ActivationFunctionType.Square
                )
                sqv = sq.rearrange("p (a b c) -> p a b c", a=2, b=2)
                s34 = mpool.tile([64, 256], F32, tag="s34")
                s34v = s34.rearrange("p (a c) -> p a c", a=2)
                nc.vector.tensor_add(s34v[:], sqv[:, :, 0], sqv[:, :, 1])
                m34 = mpool.tile([64, 256], F32, tag="m34")
                nc.scalar.activation(
                    m34[:], s34[:], mybir.ActivationFunctionType.Sqrt, scale=0.25
                )
                vt = mpool.tile([64, 128], F32R, tag="vt")
                nc.vector.tensor_add(vt[:], m34[:, 0:128], m34[:, 128:256])

                # ---- stage 3: width resize; out partitions = (smp, tI) ----
                out_all = opool.tile([128, 8 * W], BF16, tag="out_all")
                for p in range(8):
                    ps3 = ps3pool.tile([128, W], F32, tag="ps3")
                    nc.tensor.matmul(
                        ps3[:],
                        vt[:],
                        wrep_sb[:, W * p : W * p + W],
                        start=True, stop=True,
                    )
                    dst = out_all[:, W * p : W * p + W]
                    c = CFG["copy_pat"][p % len(CFG["copy_pat"])]
                    if c == "v":
                        nc.vector.tensor_copy(dst, ps3[:])
                    else:
                        nc.scalar.copy(dst, ps3[:])

                # one merged store per sample: rows 8*tI + p are 8KB contiguous
                for smp in range(2):
                    bg = 2 * bg2 + smp
                    dst = ys[bg, 0].rearrange("(t p) j -> t (p j)", t=64)
                    nc.sync.dma_start(dst, out_all[64 * smp : 64 * smp + 64, :])

            if rep_ctx is not None:
                rep_ctx.__exit__(None, None, None)

    nc.compile()
    return nc


_NC = None


def _get_program():
    global _NC
    if _NC is None:
        _NC = _build_program()
    return _NC


def _prep_inputs(x: np.ndarray) -> np.ndarray:
    """[64,3,512,512] f32 -> [64, 128, 1536] bf16 with the needed columns
    gathered and rows regrouped: out[s, p, (ch,q,c)] = x[s, ch, 128q+p, COLS[c]]."""
    xsel = x[:, :, :, _COLS]  # [64, 3, 512, 128]
    xr = xsel.reshape(B_FULL, 3, NQ, 128, 128).transpose(0, 3, 1, 2, 4)
    return np.ascontiguousarray(xr).reshape(B_FULL, 128, 3 * NQ * 128).astype(
        ml_dtypes.bfloat16
    )


def kernel(x: np.ndarray) -> np.ndarray:
    assert x.shape == (B_FULL, 3, H, W), x.shape
    x = np.ascontiguousarray(x, dtype=np.float32)
    xp = _prep_inputs(x)
    nc = _get_program()
    in_maps = []
    for c in range(N_CORES):
        in_maps.append(
            {
                "xs": xp[c * B_CORE : (c + 1) * B_CORE],
                "wred": _WRED,
                "dftc": _DFTC,
                "wrep": _WREP,
            }
        )
    res = run_bass_kernel_spmd(nc, in_maps, core_ids=list(range(N_CORES)))
    out = np.concatenate([res.results[c]["ys"] for c in range(N_CORES)], axis=0)
    return out.astype(np.float32)


def _make_in_maps(x: np.ndarray):
    xp = _prep_inputs(np.ascontiguousarray(x, dtype=np.float32))
    return [
        {
            "xs": xp[c * B_CORE : (c + 1) * B_CORE],
            "wred": _WRED,
            "dftc": _DFTC,
            "wrep": _WREP,
        }
        for c in range(N_CORES)
    ]


# revision 28
# speedup vs baseline: 2.6085x; 1.0910x over previous
"""Trainium2 Bass kernel for nn_DCTHighPass.

Reference computation (per sample, 512x512 RGB image):
  gray = 0.299 R + 0.587 G + 0.114 B
  tiles = 8x8 blocks of gray (64x64 tiles, row-major (ti, tj))
  mag = |fft2(tile)|
  (buggy mask touches only batch 3:6 / fft rows 3:6 -> never sampled below)
  img = mag tiles stacked into [4096*8, 8]
  out = bilinear_resize(img, 512, 512)

Key reduction: the height downsample (32768 -> 512, factor 64) samples only
input rows 64i+31 and 64i+32 with weight 0.5 each, i.e. fft-row 7 of tile
(ti=i//8, tj=8*(i%8)+3) and fft-row 0 of tile (ti, tj=8*(i%8)+4).  fft rows
0/7 of an 8x8 real tile need only three 8-weight row reductions of the tile
(plain sum, cos, sin), followed by an 8-point DFT along columns.  The width
upsample (8 -> 512) is a fixed [8,512] matrix.  So per output row i:
  v = 0.5*(|DFT(cos/sin rowsums of tile tj3)| + |DFT(colsum of tile tj4)|)
  out[i, :] = v @ W8
Only image columns 64p+24 .. 64p+39 (p = i%8) are ever used.

v2 layout strategy (DMA was 75% busy in v1):
  - host gathers the 128 needed columns into a dense [B, 128p, 3ch*4q*128c]
    bf16 array, so device loads are contiguous 3KB-per-partition DMAs
    (v1's strided 64B descriptors ran at ~9B/ns/engine vs 22.5 peak);
  - stage-1/2 matmuls run in bf16 (fp32 pays 4 cycles/row on PE, bf16 1);
  - stage 3 stays fp32r (free dim 512 -> 1 cycle/row, full precision);
  - output is written bf16 (halves store traffic), one merged 8KB-per-
    partition store per sample; host upcasts to fp32.

Pipeline per 2 samples (per core, batch of 8 samples):
  load  xin [128, 1536] bf16 per sample
  gray (ACT/DVE, bf16): g = .299R + .587G + .114B -> xgray [128, 2*512]
  stage1 (PE, bf16): per (smp, q): xgray chunk stationary, wred [128,48]
         moving -> ps1 [128=(p,cc), 48=(type,tI_l)] -> rhs2 bf16
  stage2 (PE, bf16): 6 block-diag DFT matmuls -> psQ [64=(p,k), 512]
  magnitude (ACT/DVE): 0.5*sqrt(re^2+im^2) sums -> vt [64, 128] fp32r
  stage3 (PE, fp32r): vt stationary @ p-masked replicated W8 -> 8x [128, 512]
  drain PSUM -> out_all [128=(smp,tI), 8p*512j] bf16 (DVE/ACT/Pool copies)
  store ys rows 8*tI+p: one [64, 4096] DMA per sample (8KB contiguous rows)
"""

import sys

sys.path.insert(0, "/opt/trn_rl_repo")

import numpy as np
import ml_dtypes

from concourse import bacc
import concourse.mybir as mybir
from concourse.tile import TileContext
from concourse.bass_utils import run_bass_kernel_spmd

N_CORES = 8
B_FULL = 64
B_CORE = B_FULL // N_CORES  # 8 samples per core
H = W = 512
K = 8  # fft tile size
NQ = 4  # 128-row chunks per image
F32 = mybir.dt.float32
BF16 = mybir.dt.bfloat16
F32R = mybir.dt.float32r

# image columns ever sampled by the width resize: 64p+24 .. 64p+39
_COLS = np.concatenate([np.arange(64 * p + 24, 64 * p + 40) for p in range(K)])


# ----------------------------------------------------------------------------
# host-side constants
# ----------------------------------------------------------------------------
def _make_constants():
    j = np.arange(K)
    cosr = np.cos(2 * np.pi * j / K)
    sinr = np.sin(2 * np.pi * j / K)

    # wred [128, 144]: free = 48*ch + 16*type + tI_l, gray coef folded in;
    # type 0: plain sum (A), 1: cos rowsum (Cr), 2: sin rowsum (Ci);
    # partition = 8*tI_l + row
    coef = [0.299, 0.587, 0.114]
    wtypes = [np.ones(K), cosr, sinr]
    wred = np.zeros((128, 144), dtype=np.float32)
    for ch in range(3):
        for ty in range(3):
            for t in range(16):
                wred[8 * t : 8 * t + 8, 48 * ch + 16 * ty + t] = (
                    coef[ch] * wtypes[ty]
                )

    # dft matrices C[v,c] = cos(2pi v c/8), S[v,c] = sin(2pi v c/8)
    v = np.arange(K)
    C8 = np.cos(2 * np.pi * np.outer(v, j) / K).astype(np.float32)
    S8 = np.sin(2 * np.pi * np.outer(v, j) / K).astype(np.float32)

    # dftc [128, 320]: 5 blocks of 64 cols: [C*g0 | S*g0 | -S*g0 | C*g1 | S*g1]
    # partition = 16p + cc (cc in 0..15, g = cc//8); out col = 64*s + 8p + k
    dftc = np.zeros((128, 320), dtype=np.float32)
    for p in range(8):
        for cc in range(16):
            g, c = divmod(cc, 8)
            for k in range(8):
                cv, sv = C8[k, c], S8[k, c]
                if g == 0:
                    dftc[16 * p + cc, 0 + 8 * p + k] = cv
                    dftc[16 * p + cc, 64 + 8 * p + k] = sv
                    dftc[16 * p + cc, 128 + 8 * p + k] = -sv
                else:
                    dftc[16 * p + cc, 192 + 8 * p + k] = cv
                    dftc[16 * p + cc, 256 + 8 * p + k] = sv

    # W8 [8, 512]: bilinear width resize 8 -> 512 (align_corners=False)
    src = (np.arange(W) + 0.5) * (K / W) - 0.5
    src = np.clip(src, 0.0, K - 1.0)
    i0 = np.floor(src).astype(np.int64)
    i1 = np.minimum(i0 + 1, K - 1)
    fr = (src - i0).astype(np.float32)
    W8 = np.zeros((K, W), dtype=np.float32)
    for jj in range(W):
        W8[i0[jj], jj] += 1.0 - fr[jj]
        W8[i1[jj], jj] += fr[jj]

    # wrep [64, 8*512]: block p holds W8 on partitions 8p..8p+7, zero elsewhere
    wrep = np.zeros((64, 8 * W), dtype=np.float32)
    for p in range(8):
        wrep[8 * p : 8 * p + 8, W * p : W * p + W] = W8

    return (
        wred.astype(ml_dtypes.bfloat16),
        dftc.astype(ml_dtypes.bfloat16),
        wrep.astype(ml_dtypes.bfloat16),
    )


_WRED, _DFTC, _WREP = _make_constants()


# ----------------------------------------------------------------------------
# bass program (identical on all cores; per-core inputs differ)
# ----------------------------------------------------------------------------
CFG = dict(xin_bufs=3, mid_bufs=3, out_bufs=3, ps1_bufs=2, ps2_bufs=2,
           ps3_bufs=3, copy_pat="avavavav", split_load=1, split_store=2)


def _build_program(repeat=1, variant="full", unroll=False):
    nold = variant in ("nold", "nodma")
    nost = variant in ("nost", "nodma")
    nc = bacc.Bacc()

    xs = nc.declare_dram_parameter("xs", [B_CORE, 128, 3 * NQ * 128], BF16, isOutput=False)
    wred_d = nc.declare_dram_parameter("wred", [128, 144], BF16, isOutput=False)
    dftc_d = nc.declare_dram_parameter("dftc", [128, 320], BF16, isOutput=False)
    wrep_d = nc.declare_dram_parameter("wrep", [64, 8 * W], BF16, isOutput=False)
    ys = nc.declare_dram_parameter("ys", [B_CORE, 1, H, W], BF16, isOutput=True)

    with TileContext(nc) as tc:
        with (
            tc.tile_pool(name="consts", bufs=1) as cpool,
            tc.tile_pool(name="xin", bufs=CFG["xin_bufs"]) as xpool,
            tc.tile_pool(name="mid", bufs=CFG["mid_bufs"]) as mpool,
            tc.tile_pool(name="outp", bufs=CFG["out_bufs"]) as opool,
            tc.tile_pool(name="ps1", bufs=CFG["ps1_bufs"], space="PSUM") as ps1pool,
            tc.tile_pool(name="ps2", bufs=CFG["ps2_bufs"], space="PSUM") as ps2pool,
            tc.tile_pool(name="ps3", bufs=CFG["ps3_bufs"], space="PSUM") as ps3pool,
        ):
            wred_sb = cpool.tile([128, 144], BF16, tag="wred")
            nc.sync.dma_start(wred_sb[:], wred_d[:])
            dftc_sb = cpool.tile([128, 320], BF16, tag="dftc")
            nc.sync.dma_start(dftc_sb[:], dftc_d[:])
            wrep_sb = cpool.tile([64, 8 * W], BF16, tag="wrep")
            nc.scalar.dma_start(wrep_sb[:], wrep_d[:])
            xconst = []
            if nold:
                # ablation: inputs loaded once, loop reads static tiles
                for smp in range(2):
                    xc = cpool.tile([128, 3 * NQ * 128], BF16, tag=f"xc{smp}")
                    nc.gpsimd.dma_start(xc[:], xs[smp])
                    xconst.append(
                        xc.rearrange("p (ch q c) -> p ch q c", ch=3, q=NQ)
                    )

            rep_ctx = tc.For_i(0, repeat, 1) if repeat > 1 and not unroll else None
            if rep_ctx is not None:
                rep_ctx.__enter__()
            n_unroll = repeat if unroll else 1
            for bg2u in range(n_unroll * (B_CORE // 2)):
                bg2 = bg2u % (B_CORE // 2)
                # ---- loads: one contiguous [128, 1536] bf16 DMA per sample
                if nold:
                    xn = xconst
                else:
                    xn = []
                    sl = CFG["split_load"]
                    for smp in range(2):
                        bg = 2 * bg2 + smp
                        xin = xpool.tile([128, 3 * NQ * 128], BF16, tag=f"xn{smp}")
                        xiv = xin.rearrange("p (s r) -> p s r", s=sl)
                        xsv = xs[bg].rearrange("p (s r) -> p s r", s=sl)
                        for s in range(sl):
                            nc.gpsimd.dma_start(xiv[:, s], xsv[:, s])
                        xn.append(
                            xin.rearrange("p (ch q c) -> p ch q c", ch=3, q=NQ)
                        )

                # ---- stage 1: gray folded into channel-accumulated row
                # reductions -> rhs2 [128, 2*192] bf16
                rhs2 = mpool.tile([128, 2 * 192], BF16, tag="rhs2")
                for smp in range(2):
                    ps1 = ps1pool.tile([128, 192], F32, tag="ps1")
                    for q in range(NQ):
                        for ch in range(3):
                            nc.tensor.matmul(
                                ps1[:, 48 * q : 48 * q + 48],
                                xn[smp][:, ch, q],
                                wred_sb[:, 48 * ch : 48 * ch + 48],
                                start=(ch == 0), stop=(ch == 2),
                            )
                    nc.vector.tensor_copy(
                        rhs2[:, 192 * smp : 192 * smp + 192], ps1[:]
                    )

                # ---- stage 2: DFT + height-blend fused via PSUM accumulation
                # psQ [64=(p,k), 512] = [R3 | I3 | R4 | I4] blocks of (smp,q,tI_l)
                rhs2v = rhs2.rearrange("p (s q blk) -> p s q blk", s=2, q=NQ)
                selA = rhs2v[:, :, :, 0:16]
                selCr = rhs2v[:, :, :, 16:32]
                selCi = rhs2v[:, :, :, 32:48]
                psQ = ps2pool.tile([64, 512], F32, tag="psQ")
                C0 = dftc_sb[:, 0:64]
                S0 = dftc_sb[:, 64:128]
                S0n = dftc_sb[:, 128:192]
                C1 = dftc_sb[:, 192:256]
                S1 = dftc_sb[:, 256:320]
                nc.tensor.matmul(psQ[:, 0:128], C0, selCr, start=True, stop=False)
                nc.tensor.matmul(psQ[:, 0:128], S0, selCi, start=False, stop=True)
                nc.tensor.matmul(psQ[:, 128:256], C0, selCi, start=True, stop=False)
                nc.tensor.matmul(psQ[:, 128:256], S0n, selCr, start=False, stop=True)
                nc.tensor.matmul(psQ[:, 256:384], C1, selA, start=True, stop=True)
                nc.tensor.matmul(psQ[:, 384:512], S1, selA, start=True, stop=True)

                # magnitudes: m = 0.5*sqrt(re^2 + im^2); psQ blocks are
                # [R3 | I3 | R4 | I4], pair-summed via strided APs
                sq = mpool.tile([64, 512], F32, tag="sq")
                nc.scalar.activation(
                    sq[:], psQ[:], mybir.ActivationFunctionType.Square
                )
                sqv = sq.rearrange("p (a b c) -> p a b c", a=2, b=2)
                s34 = mpool.tile([64, 256], F32, tag="s34")
                s34v = s34.rearrange("p (a c) -> p a c", a=2)
                nc.vector.tensor_add(s34v[:], sqv[:, :, 0], sqv[:, :, 1])
                m34 = mpool.tile([64, 256], F32, tag="m34")
                nc.scalar.activation(
                    m34[:], s34[:], mybir.ActivationFunctionType.Sqrt, scale=0.25
                )
                vt = mpool.tile([64, 128], BF16, tag="vt")
                nc.vector.tensor_add(vt[:], m34[:, 0:128], m34[:, 128:256])

                # ---- stage 3: width resize; out partitions = (smp, tI) ----
                out_all = opool.tile([128, 8 * W], BF16, tag="out_all")
                for p in range(8):
                    ps3 = ps3pool.tile([128, W], F32, tag="ps3")
                    nc.tensor.matmul(
                        ps3[:],
                        vt[:],
                        wrep_sb[:, W * p : W * p + W],
                        start=True, stop=True,
                    )
                    dst = out_all[:, W * p : W * p + W]
                    c = CFG["copy_pat"][p % len(CFG["copy_pat"])]
                    if c == "v":
                        nc.vector.tensor_copy(dst, ps3[:])
                    else:
                        nc.scalar.copy(dst, ps3[:])

                # merged stores per sample: rows 8*tI + p are contiguous in
                # (p j); split halves fire as soon as their copies land
                ss = CFG["split_store"]
                for smp in range(2) if not nost else []:
                    bg = 2 * bg2 + smp
                    dst = ys[bg, 0].rearrange(
                        "(t h p) j -> h t (p j)", t=64, h=ss
                    )
                    src = out_all[64 * smp : 64 * smp + 64, :].rearrange(
                        "t (h pj) -> t h pj", h=ss
                    )
                    for hh in range(ss):
                        nc.sync.dma_start(dst[hh], src[:, hh])

            if rep_ctx is not None:
                rep_ctx.__exit__(None, None, None)

    nc.compile()
    return nc


_NC = None


def _get_program():
    global _NC
    if _NC is None:
        _NC = _build_program()
    return _NC


def _prep_inputs(x: np.ndarray) -> np.ndarray:
    """[64,3,512,512] f32 -> [64, 128, 1536] bf16 with the needed columns
    gathered and rows regrouped: out[s, p, (ch,q,c)] = x[s, ch, 128q+p, COLS[c]]."""
    xsel = x[:, :, :, _COLS]  # [64, 3, 512, 128]
    xr = xsel.reshape(B_FULL, 3, NQ, 128, 128).transpose(0, 3, 1, 2, 4)
    return np.ascontiguousarray(xr).reshape(B_FULL, 128, 3 * NQ * 128).astype(
        ml_dtypes.bfloat16
    )


def kernel(x: np.ndarray) -> np.ndarray:
    assert x.shape == (B_FULL, 3, H, W), x.shape
    x = np.ascontiguousarray(x, dtype=np.float32)
    xp = _prep_inputs(x)
    nc = _get_program()
    in_maps = []
    for c in range(N_CORES):
        in_maps.append(
            {
                "xs": xp[c * B_CORE : (c + 1) * B_CORE],
                "wred": _WRED,
                "dftc": _DFTC,
                "wrep": _WREP,
            }
        )
    res = run_bass_kernel_spmd(nc, in_maps, core_ids=list(range(N_CORES)))
    out = np.concatenate([res.results[c]["ys"] for c in range(N_CORES)], axis=0)
    return out.astype(np.float32)


def _make_in_maps(x: np.ndarray):
    xp = _prep_inputs(np.ascontiguousarray(x, dtype=np.float32))
    return [
        {
            "xs": xp[c * B_CORE : (c + 1) * B_CORE],
            "wred": _WRED,
            "dftc": _DFTC,
            "wrep": _WREP,
        }
        for c in range(N_CORES)
    ]
